# revision 1
# baseline (speedup 1.0000x reference)
"""EVA-02 ViT attention block (LoRA + rope + rel-pos-bias) on 8 TRN2 NeuronCores.

v2 — data-parallel over batch (8 images per core). Per core:
  - LoRA merged into qkv weights on the host; q-scale and v-bias folded away.
  - all weights/activations bf16 (PSUM accumulation f32): FWL weight loads,
    half DMA, 4x DVE elementwise.
  - rel-pos bias applied as exp(rpb) multiplied into the probs on DVE
    (exp(S+rpb) = exp(S)*exp(rpb)) — no identity matmuls on PE.
  - softmax reciprocal via DVE nc.vector.reciprocal — ScalarE only ever
    runs Exp/Copy (one act table set, no reloads); sums split into two
    12-row groups (psum partitions 0/64) so each half's reciprocal and
    broadcast overlap the other half's attention matmuls.
  - NOTE (hardware): a matmul with start=True whose stationary operand
    sits at base partition 64 aborts the device (NRT exec-unit error);
    ph=1 score groups are therefore seeded by a K=1 base-0 zero matmul
    and accumulate with start=False.
  - output projection emitted transposed (free dim = tokens) so the 69-token
    remainder chunk costs nothing; proj bias added as per-partition
    activation bias during eviction; host re-transposes y.
"""
import numpy as np
import ml_dtypes

B, N, C, H, R = 64, 197, 768, 12, 24
D = C // H               # 64
NCORES = 8
BPC = B // NCORES        # images per core
F2 = 2 * N               # 394
F4 = 4 * N               # 788
N0, N1 = 128, N - 128    # token chunks: 128 + 69

_cache = {}

SHUF_MASK = list(range(16, 32)) + list(range(0, 16))


def _perm64():
    p = []
    for blk in range(2):
        base = blk * 32
        p += [base + 2 * t for t in range(16)]
        p += [base + 2 * t + 1 for t in range(16)]
    return np.array(p)


def _swap16_rows(a):
    out = np.empty_like(a)
    for s in range(a.shape[0] // 32):
        out[s * 32:s * 32 + 16] = a[s * 32 + 16:s * 32 + 32]
        out[s * 32 + 16:s * 32 + 32] = a[s * 32:s * 32 + 16]
    return out


def build_program(n_pairs=BPC // 2, repeat=1):
    import concourse.bass as bass
    import concourse.tile as tile
    from concourse import bacc, mybir

    f32, bf16 = mybir.dt.float32, mybir.dt.bfloat16
    AF = mybir.ActivationFunctionType
    OP = mybir.AluOpType

    nc = bacc.Bacc("TRN2", target_bir_lowering=False, debug=False)
    n_img = 2 * n_pairs

    xt_d = nc.dram_tensor("xt", [n_pairs, C, F2], bf16, kind="ExternalInput")
    wt_d = nc.dram_tensor("wt", [C, 3 * C], bf16, kind="ExternalInput")
    bq_d = nc.dram_tensor("bq", [128, 6], f32, kind="ExternalInput")
    cs_d = nc.dram_tensor("cs", [2, 128, F4], bf16, kind="ExternalInput")
    erpb_d = nc.dram_tensor("erpb", [12, 128, F2], bf16, kind="ExternalInput")
    projt_d = nc.dram_tensor("projt", [C, C], bf16, kind="ExternalInput")
    projb_d = nc.dram_tensor("projb", [128, 6], f32, kind="ExternalInput")
    y_d = nc.dram_tensor("y", [n_pairs, C, F2], bf16, kind="ExternalOutput")

    from contextlib import ExitStack
    with tile.TileContext(nc) as tc:
        with ExitStack() as stk:
            pool = lambda name, bufs, **kw: stk.enter_context(
                tc.tile_pool(name=name, bufs=bufs, **kw))
            # NOTE: bufs is per-tag. PSUM budget: qkps 2 + vps 1 + psA 1 +
            # psB 1 + aops 1 + sums 1 + yps 1 = 8 banks exactly.
            constp = pool("const", 1)
            xtp = pool("xt", 2)
            qkps = pool("qkps", 2, space="PSUM")
            vps = pool("vps", 1, space="PSUM")
            qkbfp = pool("qkbf", 2)
            ropet = pool("ropet", 1)
            vsbp = pool("vsb", 8)
            scps = pool("scps", 1, space="PSUM")
            yps = pool("yps", 1, space="PSUM")
            probsp = pool("probs", 4)
            aops = pool("aops", 1, space="PSUM")
            sumsp = pool("sums", 1, space="PSUM")
            rsbp = pool("rsb", 2)
            aosbp = pool("aosb", 6)
            rbc = pool("rbc", 2)
            ysbp = pool("ysb", 2)
            otp = pool("otp", 6)

            # ---- constants ----
            # small ones first on the sync queue, then per-chunk weight DMAs
            # (the first qk matmuls chase the stream); late-needed constants
            # (erpb/projt/projb) go on the gpsimd SWDGE queue.
            bq_sb = constp.tile([128, 6], f32, tag="bq")
            nc.sync.dma_start(bq_sb[:], bq_d[:])
            cos_sb = constp.tile([128, F4], bf16, tag="cos")
            nc.sync.dma_start(cos_sb[:], cs_d[0])
            spm_sb = constp.tile([128, F4], bf16, tag="spm")
            nc.sync.dma_start(spm_sb[:], cs_d[1])
            # first pair's inputs ahead of the weight stream: the first qk
            # matmul needs only xt + wt chunk 0
            xt_first = []
            for cc in range(6):
                t = xtp.tile([128, F2], bf16, tag=f"xt{cc}", name=f"xtF{cc}")
                nc.sync.dma_start(t[:], xt_d[0, cc * 128:(cc + 1) * 128, :])
                xt_first.append(t)
            wt_all = constp.tile([128, 6 * 3 * C], bf16, tag="wtall")
            for cc in range(6):
                nc.sync.dma_start(
                    wt_all[:, cc * 3 * C:(cc + 1) * 3 * C],
                    wt_d[cc * 128:(cc + 1) * 128, :])
            wt_sb = [wt_all[:, cc * 3 * C:(cc + 1) * 3 * C] for cc in range(6)]
            pt_all = constp.tile([128, 6 * C], bf16, tag="ptall")
            erpb_all = constp.tile([128, 12 * F2], bf16, tag="erpball")
            # erpb_sb[hp][jchunk] -> [128, F2] (cols: ph*N + i)
            erpb_sb = [(erpb_all[:, (hp * 2) * F2:(hp * 2 + 1) * F2],
                        erpb_all[:, (hp * 2 + 1) * F2:(hp * 2 + 2) * F2])
                       for hp in range(6)]
            projb_sb = constp.tile([128, 6], f32, tag="pb")

            def load_late_consts():
                # emitted after the first pair's xt DMAs so the first qk
                # matmuls aren't queued behind these on the sync queue
                nc.sync.dma_start(
                    pt_all[:].rearrange("p (cc j) -> p cc j", cc=6),
                    projt_d.rearrange("(cc p) j -> cc p j", cc=6)
                    .transpose((1, 0, 2)))
                nc.sync.dma_start(
                    erpb_all[:].rearrange("p (g j) -> p g j", g=12),
                    erpb_d.transpose((1, 0, 2)))
                nc.sync.dma_start(projb_sb[:], projb_d[:])
            # E-band: column 11 is ones; slicing [:, 11-h:23-h] gives a
            # [128, 12] selector with ones in column h.
            eband = constp.tile([128, 23], bf16, tag="eband")
            nc.vector.memset(eband[:], 0.0)
            nc.vector.memset(eband[:, 11:12], 1.0)
            zrow = constp.tile([1, F2], bf16, tag="zrow")
            nc.vector.memset(zrow[:], 0.0)

            qk_quad = {}
            v_pairs = {}

            def attention(p, par, xt_ref):
                """Scores/attn/normalize/proj for image pair p (quad slot par)."""
                v_sb = v_pairs.pop(p)
                ot_sb = []
                ao_list = []
                # one bank; heads 0-5 sum into rows 0:12, heads 6-11 into
                # rows 64:76 (base-64 psum writes are fine; only base-64
                # stationary operands with start=True abort) so the first
                # half's reciprocal/broadcast can start mid-pair.
                sums_ps = sumsp.tile([128, F2], f32, tag="sums",
                                     padded_shape=[128, 512], name=f"sums{p}")
                for half in range(2):
                    nc.tensor.matmul(
                        sums_ps[64 * half:64 * half + 12, :],
                        lhsT=eband[0:1, 0:12], rhs=zrow[0:1, :],
                        start=True, stop=False, skip_group_check=True)
                for hp in range(6):
                    qro = qk_quad[hp + 100]
                    kro = qk_quad[hp + 6 + 100]
                    ao = aops.tile([128, F2], f32, tag="aops",
                                   padded_shape=[128, 512], name=f"ao{p}{hp}")
                    for ic in range(2):
                        qoff = ic * N
                        psA = scps.tile([128, F2], f32, tag="psA",
                                        padded_shape=[128, 512], name=f"psA{p}{hp}{ic}")
                        psB = scps.tile([128, F2], f32, tag="psB",
                                        padded_shape=[128, 512], name=f"psB{p}{hp}{ic}")
                        for ph in range(2):
                            cr = ph * N
                            qv = qro[ph * 64:(ph + 1) * 64, qoff:qoff + N]
                            # start=True with a base-64 stationary operand
                            # aborts the device; seed ph=1 columns with a K=1
                            # base-0 zero matmul and accumulate instead.
                            if ph == 1:
                                nc.tensor.matmul(
                                    psA[:, cr:cr + N], lhsT=zrow[0:1, 0:128],
                                    rhs=zrow[0:1, cr:cr + N],
                                    start=True, stop=False)
                            nc.tensor.matmul(
                                psA[:, cr:cr + N],
                                lhsT=kro[ph * 64:(ph + 1) * 64, qoff:qoff + 128],
                                rhs=qv, start=(ph == 0), stop=True)
                            if ph == 1:
                                nc.tensor.matmul(
                                    psB[0:N1, cr:cr + N], lhsT=zrow[0:1, 0:N1],
                                    rhs=zrow[0:1, cr:cr + N],
                                    start=True, stop=False)
                            nc.tensor.matmul(
                                psB[0:N1, cr:cr + N],
                                lhsT=kro[ph * 64:(ph + 1) * 64, qoff + 128:qoff + N],
                                rhs=qv, start=(ph == 0), stop=True)
                        prA = probsp.tile([128, F2], bf16, tag="prA",
                                          name=f"prA{p}{hp}{ic}")
                        prB = probsp.tile([128, F2], bf16, tag="prB",
                                          name=f"prB{p}{hp}{ic}")
                        nc.scalar.activation(prA[:], psA[:], AF.Exp)
                        nc.scalar.activation(prB[0:N1, :], psB[0:N1, :], AF.Exp)
                        # fold in the rel-pos bias: exp(S)*exp(rpb)
                        nc.vector.tensor_mul(prA[:], prA[:], erpb_sb[hp][0])
                        nc.vector.tensor_mul(prB[0:N1, :], prB[0:N1, :],
                                             erpb_sb[hp][1][0:N1, :])
                        for ph in range(2):
                            h = 2 * hp + ph
                            cr = ph * N
                            nc.tensor.matmul(
                                ao[ph * 64:(ph + 1) * 64, ic * N:(ic + 1) * N],
                                lhsT=v_sb[ic][0][:, h * 64:(h + 1) * 64],
                                rhs=prA[:, cr:cr + N], start=True, stop=False)
                            nc.tensor.matmul(
                                ao[ph * 64:(ph + 1) * 64, ic * N:(ic + 1) * N],
                                lhsT=v_sb[ic][1][0:N1, h * 64:(h + 1) * 64],
                                rhs=prB[0:N1, cr:cr + N], start=False, stop=True)
                            half = hp // 3
                            hh = h - 6 * half
                            srows = sums_ps[64 * half:64 * half + 12,
                                            ic * N:(ic + 1) * N]
                            last = (hp % 3 == 2 and ph == 1)
                            nc.tensor.matmul(
                                srows, lhsT=eband[:, 11 - hh:23 - hh],
                                rhs=prA[:, cr:cr + N],
                                start=False, stop=False, skip_group_check=True)
                            nc.tensor.matmul(
                                srows, lhsT=eband[0:N1, 11 - hh:23 - hh],
                                rhs=prB[0:N1, cr:cr + N],
                                start=False, stop=last, skip_group_check=True)
                    aot = aosbp.tile([128, F2], bf16, tag="aosb",
                                     name=f"aot{p}{hp}")
                    nc.scalar.activation(aot[:], ao[:], AF.Copy)
                    ao_list.append(aot)
                    if hp % 3 == 2:
                        # ---- half-normalization: r = 1/sums on DVE ----
                        half = hp // 3
                        rsm = rsbp.tile([6, F2], f32, tag="rsm",
                                        name=f"rsm{p}{half}")
                        nc.vector.reciprocal(
                            rsm[:], sums_ps[64 * half:64 * half + 6, :])
                        rsmb = rsbp.tile([6, F2], bf16, tag="rsmb",
                                         name=f"rsmb{p}{half}")
                        nc.vector.tensor_copy(rsmb[:], rsm[:])
                        for hq in range(3 * half, 3 * half + 3):
                            r0 = 2 * hq - 6 * half
                            rb = rbc.tile([128, F2], bf16, tag="rbc",
                                          name=f"rb{p}{hq}")
                            nc.sync.dma_start(
                                rb[0:64, :],
                                rsmb[r0:r0 + 1, :].unsqueeze(1)
                                .broadcast_to((1, 64, F2)))
                            nc.sync.dma_start(
                                rb[64:128, :],
                                rsmb[r0 + 1:r0 + 2, :].unsqueeze(1)
                                .broadcast_to((1, 64, F2)))
                            ot = otp.tile([128, F2], bf16, tag="ot",
                                          name=f"ot{p}{hq}")
                            nc.vector.tensor_mul(ot[:], ao_list[hq][:], rb[:])
                            ot_sb.append(ot)

                # ---- output projection, transposed (free dim = tokens) ----
                for oc in range(6):
                    ps = yps.tile([128, F2], f32, tag="yps",
                                  padded_shape=[128, 512], name=f"yps{p}{oc}")
                    for cc in range(6):
                        nc.tensor.matmul(
                            ps[:],
                            lhsT=pt_all[:, cc * C + oc * 128:cc * C + (oc + 1) * 128],
                            rhs=ot_sb[cc][:],
                            start=(cc == 0), stop=(cc == 5))
                    yt = ysbp.tile([128, F2], bf16, tag="ysb",
                                   name=f"yt{p}{oc}")
                    nc.vector.tensor_scalar_add(yt[:], ps[:],
                                                projb_sb[:, oc:oc + 1])
                    nc.sync.dma_start(
                        y_d[p % n_pairs, oc * 128:(oc + 1) * 128, :], yt[:])

            total_pairs = repeat * n_pairs
            for pi in range(total_pairs):
                p = pi % n_pairs
                par = pi % 2
                # ---- load xT for this image pair ----
                if pi == 0:
                    xt_sb = xt_first
                    load_late_consts()
                else:
                    xt_sb = []
                    for cc in range(6):
                        t = xtp.tile([128, F2], bf16, tag=f"xt{cc}",
                                     name=f"xt{pi}{cc}")
                        nc.sync.dma_start(
                            t[:], xt_d[p, cc * 128:(cc + 1) * 128, :])
                        xt_sb.append(t)

                # ---- q/k projection into per-pair tiles ----
                for m in range(12):
                    qk_quad[m] = qkbfp.tile(
                        [128, F2], bf16, tag=f"qk{m}", name=f"qk{pi}{m}")
                for m in range(12):
                    ps = qkps.tile([128, F2], f32, tag="qkps",
                                   padded_shape=[128, 512], name=f"qkp{pi}{m}")
                    for cc in range(6):
                        nc.tensor.matmul(
                            ps[:],
                            lhsT=wt_sb[cc][:, m * 128:(m + 1) * 128],
                            rhs=xt_sb[cc][:],
                            start=(cc == 0), stop=(cc == 5))
                    dst = qk_quad[m][:]
                    if m < 6:
                        # fold the q bias in here: shuffle(q+bq) = qs+bqs, so
                        # the rope needs no scalar_tensor_tensor ops at all
                        nc.vector.tensor_scalar_add(dst, ps[:],
                                                    bq_sb[:, m:m + 1])
                    else:
                        nc.scalar.activation(dst, ps[:], AF.Copy)

                # ---- v projection (natural out) ----
                v_sb = []
                for ic in range(2):
                    vts = [vsbp.tile([128, C], bf16, tag="vsb",
                                     name=f"vsb{pi}{ic}{i}") for i in range(2)]
                    for nck, (n_off, n_sz) in enumerate(((0, N0), (N0, N1))):
                        for ch in range(2):
                            ps = vps.tile([128, 384], f32, tag="vps",
                                          padded_shape=[128, 512],
                                          name=f"vps{pi}{ic}{nck}{ch}")
                            for cc in range(6):
                                nc.tensor.matmul(
                                    ps[0:n_sz, :],
                                    lhsT=xt_sb[cc][:, ic * N + n_off:ic * N + n_off + n_sz],
                                    rhs=wt_sb[cc][:, 2 * C + ch * 384:2 * C + (ch + 1) * 384],
                                    start=(cc == 0), stop=(cc == 5))
                            nc.scalar.activation(
                                vts[nck][0:n_sz, ch * 384:(ch + 1) * 384],
                                ps[0:n_sz, :], AF.Copy)
                    v_sb.append(vts)
                v_pairs[p] = v_sb

                # ---- rope, then attention for this pair ----
                for m in range(12):
                    src = qk_quad[m]
                    qs = ropet.tile([128, F2], bf16, tag="qs", name=f"qs{pi}{m}")
                    nc.vector.stream_shuffle(
                        qs[:].bitcast(f32), src[:].bitcast(f32), SHUF_MASK)
                    u = ropet.tile([128, F2], bf16, tag="u", name=f"u{pi}{m}")
                    v = ropet.tile([128, F2], bf16, tag="v", name=f"v{pi}{m}")
                    nc.vector.tensor_mul(u[:], src[:], cos_sb[:, 0:F2])
                    nc.vector.tensor_mul(v[:], qs[:], spm_sb[:, 0:F2])
                    nc.vector.tensor_add(src[:], u[:], v[:])
                    qk_quad[m + 100] = src
                attention(p, 0, None)
    nc.compile()
    return nc


def host_prepare(inputs):
    x = np.asarray(inputs["x"], np.float32)
    qkv_w = np.asarray(inputs["qkv_w"], np.float32)
    scale = D ** -0.5
    Wq = qkv_w[:C] + np.asarray(inputs["lora_q_b"]) @ np.asarray(inputs["lora_q_a"])
    Wk = qkv_w[C:2 * C] + np.asarray(inputs["lora_k_b"]) @ np.asarray(inputs["lora_k_a"])
    Wv = qkv_w[2 * C:] + np.asarray(inputs["lora_v_b"]) @ np.asarray(inputs["lora_v_a"])
    p64 = _perm64()
    perm = (np.arange(H)[:, None] * D + p64[None, :]).ravel()
    Wq_de = (Wq * scale)[perm]
    bq_de = (np.asarray(inputs["q_bias"], np.float32) * scale)[perm]
    Wk_de = Wk[perm]
    wt = np.ascontiguousarray(
        np.concatenate([Wq_de, Wk_de, Wv], 0).T).astype(ml_dtypes.bfloat16)

    bq = np.ascontiguousarray(bq_de.reshape(6, 128).T)

    cos_f = np.ones((N, D), np.float32)
    cos_f[1:] = np.asarray(inputs["rope_cos"], np.float32)
    sin_f = np.zeros((N, D), np.float32)
    sin_f[1:] = np.asarray(inputs["rope_sin"], np.float32)
    cos_de = np.ascontiguousarray(cos_f[:, p64].T)
    spm = np.ascontiguousarray(sin_f[:, p64].T)
    for blk in range(2):
        spm[blk * 32:blk * 32 + 16] *= -1.0
    cs = np.stack([
        np.tile(np.vstack([cos_de, cos_de]), (1, 4)),
        np.tile(np.vstack([spm, spm]), (1, 4)),
    ]).astype(ml_dtypes.bfloat16)

    rel_table = np.asarray(inputs["rel_table"], np.float32)
    rel_index = np.asarray(inputs["rel_index"])
    rpb = rel_table[rel_index.reshape(-1)].reshape(N, N, H)
    rpbT = np.exp(rpb.transpose(2, 1, 0))  # [h, j, i] -> exp
    # erpb[(hp, jchunk), j_local, ph*N + i]
    erpb = np.zeros((12, 128, F2), np.float32)
    for hp in range(6):
        for jc in range(2):
            jn = 128 if jc == 0 else N1
            for ph in range(2):
                erpb[hp * 2 + jc, 0:jn, ph * N:(ph + 1) * N] = \
                    rpbT[2 * hp + ph, jc * 128:jc * 128 + jn, :]
    erpb = erpb.astype(ml_dtypes.bfloat16)

    proj_w = np.asarray(inputs["proj_w"], np.float32)
    projt = np.ascontiguousarray(proj_w.T).astype(ml_dtypes.bfloat16)
    projb_full = (np.asarray(inputs["proj_b"], np.float32)
                  + proj_w @ np.asarray(inputs["v_bias"], np.float32))
    projb = np.ascontiguousarray(projb_full.reshape(6, 128).T)

    xt = x.transpose(0, 2, 1)  # [B, C, N]
    xt_pairs = np.ascontiguousarray(
        xt.reshape(B // 2, 2, C, N).transpose(0, 2, 1, 3)
        .reshape(B // 2, C, 2 * N)).astype(ml_dtypes.bfloat16)

    shared = dict(wt=wt, bq=bq, cs=cs, erpb=erpb,
                  projt=projt, projb=projb)
    per_core = []
    ppc = BPC // 2
    for c in range(NCORES):
        m = dict(shared)
        m["xt"] = np.ascontiguousarray(xt_pairs[c * ppc:(c + 1) * ppc])
        per_core.append(m)
    return per_core


def postprocess(y_all):
    """y_all: [..., C, F2] pair-major; returns [n_images, N, C]."""
    y = np.asarray(y_all, np.float32).reshape(-1, C, 2, N)
    return np.ascontiguousarray(
        y.transpose(0, 2, 3, 1).reshape(-1, N, C))


def kernel(**inputs):
    from concourse.bass_utils import run_bass_kernel_spmd
    in_maps = host_prepare(inputs)
    if "nc" not in _cache:
        _cache["nc"] = build_program()
    nc = _cache["nc"]
    res = run_bass_kernel_spmd(nc, in_maps, list(range(NCORES))).results
    y = np.concatenate([res[c]["y"] for c in range(NCORES)], 0)
    return postprocess(y)



# revision 5
# speedup vs baseline: 1.0568x; 1.0568x over previous
"""EVA-02 ViT attention block (LoRA + rope + rel-pos-bias) on 8 TRN2 NeuronCores.

v3 — data-parallel over batch (8 images per core). Per core:
  - LoRA merged into qkv weights on the host; q-scale and v-bias folded away.
  - all weights/activations bf16 (PSUM accumulation f32).
  - rel-pos bias applied as exp(rpb) multiplied into the probs on DVE.
  - softmax denominator comes FREE from the attention matmul: v tiles carry a
    ones-column per head (stride-65 layout), so attn@V psum row 64 accumulates
    sum_k(pr) — no eband/sums matmuls on PE at all (~10% PE saved vs v2).
  - attn@V output is per-head [65, F2] psum; evicted to bf16 on the (otherwise
    idle) GPSIMD/Pool engine; sums rows gathered by one SBUF->SBUF DMA per
    half, reciprocal on DVE, broadcast via DMA, normalize muls on DVE with
    base-64 outputs (walrus only constrains INPUT partition bases to match,
    and only when both inputs are SBUF).
  - NOTE (hardware): a matmul whose stationary operand sits at base partition
    64 aborts the device unless its output bytes were already written by a
    prior matmul (zero-region pending state). ph=1 score groups are therefore
    seeded by a K=1 zero matmul; base-0 stationary groups need no seed even
    when targeting untouched bytes of a started bank.
  - q-bias folded into the psum->sbuf eviction on ACT (Identity + per-partition
    bias AP; same act table set as Exp, no table reloads).
  - output projection emitted transposed (free dim = tokens); proj bias added
    during eviction; host re-transposes y.
"""
import numpy as np
import ml_dtypes

B, N, C, H, R = 64, 197, 768, 12, 24
D = C // H               # 64
NCORES = 8
BPC = B // NCORES        # images per core
F2 = 2 * N               # 394
F4 = 4 * N               # 788
N0, N1 = 128, N - 128    # token chunks: 128 + 69
VW = 65                  # per-head v block: 64 channels + ones column
VC = 12 * VW             # 780

_cache = {}

SHUF_MASK = list(range(16, 32)) + list(range(0, 16))


def _perm64():
    p = []
    for blk in range(2):
        base = blk * 32
        p += [base + 2 * t for t in range(16)]
        p += [base + 2 * t + 1 for t in range(16)]
    return np.array(p)


def build_program(n_pairs=BPC // 2, repeat=1):
    import concourse.bass as bass
    import concourse.tile as tile
    from concourse import bacc, mybir

    f32, bf16 = mybir.dt.float32, mybir.dt.bfloat16
    AF = mybir.ActivationFunctionType
    OP = mybir.AluOpType

    nc = bacc.Bacc("TRN2", target_bir_lowering=False, debug=False)
    n_img = 2 * n_pairs

    xt_d = nc.dram_tensor("xt", [n_pairs, C, F2], bf16, kind="ExternalInput")
    wt_d = nc.dram_tensor("wt", [C, 3 * C], bf16, kind="ExternalInput")
    bq_d = nc.dram_tensor("bq", [128, 6], f32, kind="ExternalInput")
    cs_d = nc.dram_tensor("cs", [2, 128, F4], bf16, kind="ExternalInput")
    erpb_d = nc.dram_tensor("erpb", [12, 128, F2], bf16, kind="ExternalInput")
    projt_d = nc.dram_tensor("projt", [C, C], bf16, kind="ExternalInput")
    projb_d = nc.dram_tensor("projb", [128, 6], f32, kind="ExternalInput")
    y_d = nc.dram_tensor("y", [n_pairs, C, F2], bf16, kind="ExternalOutput")

    from contextlib import ExitStack
    with tile.TileContext(nc) as tc:
        with ExitStack() as stk:
            pool = lambda name, bufs, **kw: stk.enter_context(
                tc.tile_pool(name=name, bufs=bufs, **kw))
            # NOTE: bufs is per-tag. PSUM budget: qkps 2 + vps 1 + psA 1 +
            # psB 1 + aops 2 + yps 1 = 8 banks exactly.
            constp = pool("const", 1)
            xtp = pool("xt", 2)
            qkps = pool("qkps", 2, space="PSUM")
            vps = pool("vps", 1, space="PSUM")
            qkbfp = pool("qkbf", 2)
            ropet = pool("ropet", 1)
            vsbp = pool("vsb", 8)
            scps = pool("scps", 1, space="PSUM")
            yps = pool("yps", 1, space="PSUM")
            probsp = pool("probs", 4)
            aops = pool("aops", 2, space="PSUM")
            rsbp = pool("rsb", 2)
            aotp = pool("aot", 2)
            rbc = pool("rbc", 8)
            ysbp = pool("ysb", 2)
            otp = pool("otp", 6)

            # ---- constants ----
            bq_sb = constp.tile([128, 6], f32, tag="bq")
            nc.sync.dma_start(bq_sb[:], bq_d[:])
            cos_sb = constp.tile([128, F4], bf16, tag="cos")
            nc.sync.dma_start(cos_sb[:], cs_d[0])
            spm_sb = constp.tile([128, F4], bf16, tag="spm")
            nc.sync.dma_start(spm_sb[:], cs_d[1])
            # first pair's inputs ahead of the weight stream
            xt_first = []
            for cc in range(6):
                t = xtp.tile([128, F2], bf16, tag=f"xt{cc}", name=f"xtF{cc}")
                nc.sync.dma_start(t[:], xt_d[0, cc * 128:(cc + 1) * 128, :])
                xt_first.append(t)
            wt_all = constp.tile([128, 6 * 3 * C], bf16, tag="wtall")
            for cc in range(6):
                nc.sync.dma_start(
                    wt_all[:, cc * 3 * C:(cc + 1) * 3 * C],
                    wt_d[cc * 128:(cc + 1) * 128, :])
            wt_sb = [wt_all[:, cc * 3 * C:(cc + 1) * 3 * C] for cc in range(6)]
            pt_all = constp.tile([128, 6 * C], bf16, tag="ptall")
            erpb_all = constp.tile([128, 12 * F2], bf16, tag="erpball")
            # erpb_sb[hp][jchunk] -> [128, F2] (cols: ph*N + i)
            erpb_sb = [(erpb_all[:, (hp * 2) * F2:(hp * 2 + 1) * F2],
                        erpb_all[:, (hp * 2 + 1) * F2:(hp * 2 + 2) * F2])
                       for hp in range(6)]
            projb_sb = constp.tile([128, 6], f32, tag="pb")

            def load_late_consts():
                nc.sync.dma_start(
                    pt_all[:].rearrange("p (cc j) -> p cc j", cc=6),
                    projt_d.rearrange("(cc p) j -> cc p j", cc=6)
                    .transpose((1, 0, 2)))
                nc.sync.dma_start(
                    erpb_all[:].rearrange("p (g j) -> p g j", g=12),
                    erpb_d.transpose((1, 0, 2)))
                nc.sync.dma_start(projb_sb[:], projb_d[:])
            zrow = constp.tile([1, F2], bf16, tag="zrow")
            nc.vector.memset(zrow[:], 0.0)

            qk_quad = {}
            v_pairs = {}

            def attention(p, xt_ref):
                """Scores/attn/normalize/proj for image pair p."""
                v_sb = v_pairs.pop(p)
                aot = aotp.tile([65, 12 * F2], bf16, tag="aot", name=f"aot{p}")
                ot_sb = []
                for hp in range(6):
                    qro = qk_quad[hp + 100]
                    kro = qk_quad[hp + 6 + 100]
                    ao_t = [None, None]
                    for ic in range(2):
                        qoff = ic * N
                        psA = scps.tile([128, F2], f32, tag="psA",
                                        padded_shape=[128, 512], name=f"psA{p}{hp}{ic}")
                        psB = scps.tile([128, F2], f32, tag="psB",
                                        padded_shape=[128, 512], name=f"psB{p}{hp}{ic}")
                        for ph in range(2):
                            cr = ph * N
                            qv = qro[ph * 64:(ph + 1) * 64, qoff:qoff + N]
                            # base-64 stationary matmuls abort unless their
                            # psum bytes were already matmul-written: seed.
                            if ph == 1:
                                nc.tensor.matmul(
                                    psA[:, cr:cr + N], lhsT=zrow[0:1, 0:128],
                                    rhs=zrow[0:1, cr:cr + N],
                                    start=True, stop=False)
                            nc.tensor.matmul(
                                psA[:, cr:cr + N],
                                lhsT=kro[ph * 64:(ph + 1) * 64, qoff:qoff + 128],
                                rhs=qv, start=(ph == 0), stop=True)
                            if ph == 1:
                                nc.tensor.matmul(
                                    psB[0:N1, cr:cr + N], lhsT=zrow[0:1, 0:N1],
                                    rhs=zrow[0:1, cr:cr + N],
                                    start=True, stop=False)
                            nc.tensor.matmul(
                                psB[0:N1, cr:cr + N],
                                lhsT=kro[ph * 64:(ph + 1) * 64, qoff + 128:qoff + N],
                                rhs=qv, start=(ph == 0), stop=True)
                        prA = probsp.tile([128, F2], bf16, tag="prA",
                                          name=f"prA{p}{hp}{ic}")
                        prB = probsp.tile([128, F2], bf16, tag="prB",
                                          name=f"prB{p}{hp}{ic}")
                        nc.scalar.activation(prA[:], psA[:], AF.Exp)
                        nc.scalar.activation(prB[0:N1, :], psB[0:N1, :], AF.Exp)
                        # fold in the rel-pos bias: exp(S)*exp(rpb)
                        nc.vector.tensor_mul(prA[:], prA[:], erpb_sb[hp][0])
                        nc.vector.tensor_mul(prB[0:N1, :], prB[0:N1, :],
                                             erpb_sb[hp][1][0:N1, :])
                        for ph in range(2):
                            h = 2 * hp + ph
                            cr = ph * N
                            if ic == 0:
                                ao_t[ph] = aops.tile(
                                    [65, F2], f32, tag="aops",
                                    padded_shape=[65, 512], name=f"ao{p}{h}")
                            # ones column at v block col 64 accumulates the
                            # softmax denominator into psum row 64 for free
                            nc.tensor.matmul(
                                ao_t[ph][:, ic * N:(ic + 1) * N],
                                lhsT=v_sb[ic][0][:, h * VW:(h + 1) * VW],
                                rhs=prA[:, cr:cr + N],
                                start=(ic == 0), stop=False,
                                skip_group_check=True)
                            nc.tensor.matmul(
                                ao_t[ph][:, ic * N:(ic + 1) * N],
                                lhsT=v_sb[ic][1][0:N1, h * VW:(h + 1) * VW],
                                rhs=prB[0:N1, cr:cr + N],
                                start=False, stop=(ic == 1),
                                skip_group_check=True)
                    # evict both heads (psum -> sbuf; GPSIMD can't read PSUM)
                    for ph in range(2):
                        h = 2 * hp + ph
                        nc.scalar.activation(
                            aot[:, h * F2:(h + 1) * F2], ao_t[ph][:], AF.Copy)
                    if hp % 3 == 2:
                        # ---- half-normalization ----
                        half = hp // 3
                        srows = rsbp.tile([6, F2], bf16, tag="srows",
                                          name=f"srows{p}{half}")
                        nc.sync.dma_start(
                            srows[:],
                            aot[64:65, half * 6 * F2:(half + 1) * 6 * F2]
                            .rearrange("p (h n) -> p h n", h=6))
                        rsm = rsbp.tile([6, F2], f32, tag="rsm",
                                        name=f"rsm{p}{half}")
                        nc.vector.reciprocal(rsm[:], srows[:])
                        rsmb = rsbp.tile([6, F2], bf16, tag="rsmb",
                                         name=f"rsmb{p}{half}")
                        nc.vector.tensor_copy(rsmb[:], rsm[:])
                        for t in range(3 * half, 3 * half + 3):
                            j0 = 2 * t - 6 * half
                            rb0 = rbc.tile([64, F2], bf16, tag="rbc",
                                           name=f"rb0{p}{t}")
                            rb1 = rbc.tile([64, F2], bf16, tag="rbc",
                                           name=f"rb1{p}{t}")
                            nc.sync.dma_start(
                                rb0[:],
                                rsmb[j0:j0 + 1, :].unsqueeze(1)
                                .broadcast_to((1, 64, F2)))
                            nc.sync.dma_start(
                                rb1[:],
                                rsmb[j0 + 1:j0 + 2, :].unsqueeze(1)
                                .broadcast_to((1, 64, F2)))
                            ot = otp.tile([128, F2], bf16, tag="ot",
                                          name=f"ot{p}{t}")
                            # normalize on the idle Pool engine (all-SBUF)
                            nc.gpsimd.tensor_mul(
                                ot[0:64, :],
                                aot[0:64, (2 * t) * F2:(2 * t + 1) * F2],
                                rb0[:])
                            nc.gpsimd.tensor_mul(
                                ot[64:128, :],
                                aot[0:64, (2 * t + 1) * F2:(2 * t + 2) * F2],
                                rb1[:])
                            ot_sb.append(ot)

                # ---- output projection, transposed (free dim = tokens) ----
                for oc in range(6):
                    ps = yps.tile([128, F2], f32, tag="yps",
                                  padded_shape=[128, 512], name=f"yps{p}{oc}")
                    for cc in range(6):
                        nc.tensor.matmul(
                            ps[:],
                            lhsT=pt_all[:, cc * C + oc * 128:cc * C + (oc + 1) * 128],
                            rhs=ot_sb[cc][:],
                            start=(cc == 0), stop=(cc == 5))
                    yt = ysbp.tile([128, F2], bf16, tag="ysb",
                                   name=f"yt{p}{oc}")
                    nc.vector.tensor_scalar_add(yt[:], ps[:],
                                                projb_sb[:, oc:oc + 1])
                    nc.sync.dma_start(
                        y_d[p % n_pairs, oc * 128:(oc + 1) * 128, :], yt[:])

            total_pairs = repeat * n_pairs
            for pi in range(total_pairs):
                p = pi % n_pairs
                # ---- load xT for this image pair ----
                if pi == 0:
                    xt_sb = xt_first
                    load_late_consts()
                else:
                    xt_sb = []
                    for cc in range(6):
                        t = xtp.tile([128, F2], bf16, tag=f"xt{cc}",
                                     name=f"xt{pi}{cc}")
                        nc.sync.dma_start(
                            t[:], xt_d[p, cc * 128:(cc + 1) * 128, :])
                        xt_sb.append(t)

                # ---- q/k projection into per-pair tiles ----
                for m in range(12):
                    qk_quad[m] = qkbfp.tile(
                        [128, F2], bf16, tag=f"qk{m}", name=f"qk{pi}{m}")
                for m in range(12):
                    ps = qkps.tile([128, F2], f32, tag="qkps",
                                   padded_shape=[128, 512], name=f"qkp{pi}{m}")
                    for cc in range(6):
                        nc.tensor.matmul(
                            ps[:],
                            lhsT=wt_sb[cc][:, m * 128:(m + 1) * 128],
                            rhs=xt_sb[cc][:],
                            start=(cc == 0), stop=(cc == 5))
                    dst = qk_quad[m][:]
                    if m < 6:
                        # fold the q bias into the ACT eviction (Identity
                        # allows a per-partition bias AP; same table as Exp)
                        nc.scalar.activation(dst, ps[:], AF.Identity,
                                             bias=bq_sb[:, m:m + 1])
                    else:
                        nc.scalar.activation(dst, ps[:], AF.Copy)

                # ---- v projection (natural out, stride-65 head layout) ----
                v_sb = []
                for ic in range(2):
                    vts = [vsbp.tile([128, VC], bf16, tag="vsb",
                                     name=f"vsb{pi}{ic}{i}") for i in range(2)]
                    for i in range(2):
                        nc.gpsimd.memset(
                            vts[i][:].rearrange("p (h c) -> p h c", c=VW)
                            [:, :, 64:65], 1.0)
                    for nck, (n_off, n_sz) in enumerate(((0, N0), (N0, N1))):
                        for ch in range(2):
                            ps = vps.tile([128, 384], f32, tag="vps",
                                          padded_shape=[128, 512],
                                          name=f"vps{pi}{ic}{nck}{ch}")
                            for cc in range(6):
                                nc.tensor.matmul(
                                    ps[0:n_sz, :],
                                    lhsT=xt_sb[cc][:, ic * N + n_off:ic * N + n_off + n_sz],
                                    rhs=wt_sb[cc][:, 2 * C + ch * 384:2 * C + (ch + 1) * 384],
                                    start=(cc == 0), stop=(cc == 5))
                            nc.scalar.activation(
                                vts[nck][0:n_sz, ch * 6 * VW:(ch + 1) * 6 * VW]
                                .rearrange("p (h c) -> p h c", c=VW)[:, :, 0:64],
                                ps[0:n_sz, :].rearrange("p (h c) -> p h c", c=64),
                                AF.Copy)
                    v_sb.append(vts)
                v_pairs[p] = v_sb

                # ---- rope, then attention for this pair ----
                for m in range(12):
                    src = qk_quad[m]
                    qs = ropet.tile([128, F2], bf16, tag="qs", name=f"qs{pi}{m}")
                    nc.vector.stream_shuffle(
                        qs[:].bitcast(f32), src[:].bitcast(f32), SHUF_MASK)
                    u = ropet.tile([128, F2], bf16, tag="u", name=f"u{pi}{m}")
                    v = ropet.tile([128, F2], bf16, tag="v", name=f"v{pi}{m}")
                    nc.vector.tensor_mul(u[:], src[:], cos_sb[:, 0:F2])
                    nc.vector.tensor_mul(v[:], qs[:], spm_sb[:, 0:F2])
                    nc.vector.tensor_add(src[:], u[:], v[:])
                    qk_quad[m + 100] = src
                attention(p, None)
    nc.compile()
    return nc


def host_prepare(inputs):
    x = np.asarray(inputs["x"], np.float32)
    qkv_w = np.asarray(inputs["qkv_w"], np.float32)
    scale = D ** -0.5
    Wq = qkv_w[:C] + np.asarray(inputs["lora_q_b"]) @ np.asarray(inputs["lora_q_a"])
    Wk = qkv_w[C:2 * C] + np.asarray(inputs["lora_k_b"]) @ np.asarray(inputs["lora_k_a"])
    Wv = qkv_w[2 * C:] + np.asarray(inputs["lora_v_b"]) @ np.asarray(inputs["lora_v_a"])
    p64 = _perm64()
    perm = (np.arange(H)[:, None] * D + p64[None, :]).ravel()
    Wq_de = (Wq * scale)[perm]
    bq_de = (np.asarray(inputs["q_bias"], np.float32) * scale)[perm]
    Wk_de = Wk[perm]
    wt = np.ascontiguousarray(
        np.concatenate([Wq_de, Wk_de, Wv], 0).T).astype(ml_dtypes.bfloat16)

    bq = np.ascontiguousarray(bq_de.reshape(6, 128).T)

    cos_f = np.ones((N, D), np.float32)
    cos_f[1:] = np.asarray(inputs["rope_cos"], np.float32)
    sin_f = np.zeros((N, D), np.float32)
    sin_f[1:] = np.asarray(inputs["rope_sin"], np.float32)
    cos_de = np.ascontiguousarray(cos_f[:, p64].T)
    spm = np.ascontiguousarray(sin_f[:, p64].T)
    for blk in range(2):
        spm[blk * 32:blk * 32 + 16] *= -1.0
    cs = np.stack([
        np.tile(np.vstack([cos_de, cos_de]), (1, 4)),
        np.tile(np.vstack([spm, spm]), (1, 4)),
    ]).astype(ml_dtypes.bfloat16)

    rel_table = np.asarray(inputs["rel_table"], np.float32)
    rel_index = np.asarray(inputs["rel_index"])
    rpb = rel_table[rel_index.reshape(-1)].reshape(N, N, H)
    rpbT = np.exp(rpb.transpose(2, 1, 0))  # [h, j, i] -> exp
    # erpb[(hp, jchunk), j_local, ph*N + i]
    erpb = np.zeros((12, 128, F2), np.float32)
    for hp in range(6):
        for jc in range(2):
            jn = 128 if jc == 0 else N1
            for ph in range(2):
                erpb[hp * 2 + jc, 0:jn, ph * N:(ph + 1) * N] = \
                    rpbT[2 * hp + ph, jc * 128:jc * 128 + jn, :]
    erpb = erpb.astype(ml_dtypes.bfloat16)

    proj_w = np.asarray(inputs["proj_w"], np.float32)
    projt = np.ascontiguousarray(proj_w.T).astype(ml_dtypes.bfloat16)
    projb_full = (np.asarray(inputs["proj_b"], np.float32)
                  + proj_w @ np.asarray(inputs["v_bias"], np.float32))
    projb = np.ascontiguousarray(projb_full.reshape(6, 128).T)

    xt = x.transpose(0, 2, 1)  # [B, C, N]
    xt_pairs = np.ascontiguousarray(
        xt.reshape(B // 2, 2, C, N).transpose(0, 2, 1, 3)
        .reshape(B // 2, C, 2 * N)).astype(ml_dtypes.bfloat16)

    shared = dict(wt=wt, bq=bq, cs=cs, erpb=erpb,
                  projt=projt, projb=projb)
    per_core = []
    ppc = BPC // 2
    for c in range(NCORES):
        m = dict(shared)
        m["xt"] = np.ascontiguousarray(xt_pairs[c * ppc:(c + 1) * ppc])
        per_core.append(m)
    return per_core


def postprocess(y_all):
    """y_all: [..., C, F2] pair-major; returns [n_images, N, C]."""
    y = np.asarray(y_all, np.float32).reshape(-1, C, 2, N)
    return np.ascontiguousarray(
        y.transpose(0, 2, 3, 1).reshape(-1, N, C))


def kernel(**inputs):
    from concourse.bass_utils import run_bass_kernel_spmd
    in_maps = host_prepare(inputs)
    if "nc" not in _cache:
        _cache["nc"] = build_program()
    nc = _cache["nc"]
    res = run_bass_kernel_spmd(nc, in_maps, list(range(NCORES))).results
    y = np.concatenate([res[c]["y"] for c in range(NCORES)], 0)
    return postprocess(y)


# revision 21
# speedup vs baseline: 2.1017x; 1.9887x over previous
"""EVA-02 ViT attention block (LoRA + rope + rel-pos-bias) on 8 TRN2 NeuronCores.

v5 — data-parallel over batch (8 images per core). Per core:
  - LoRA merged into qkv weights on the host; q-scale and v-bias folded away.
  - all weights/activations bf16 (PSUM accumulation f32). fp8 DoubleRow was
    tried and measured ~2.6x SLOWER than bf16 on real HW (weight-load bound),
    despite the cost model predicting 2x faster - keep bf16.
  - rel-pos bias added into the score psum by identity matmuls (start=True),
    which also makes every later group's bytes matmul-written so the base-64
    stationary score matmuls need no 197-col zero seeds - only a K=1 spacer
    (a base-64 stationary load directly after a 64-row base-0 load aborts
    the device; a K=1 load between them is sufficient).
  - softmax denominator comes FREE from the attention matmul: v tiles carry a
    ones-column per head (stride-65 layout), so attn@V psum row 64 accumulates
    sum_k(pr) - no sums matmuls at all.
  - per-head normalize: reciprocal of the psum sums row straight to bf16
    (DVE), broadcast by DMA on the sync queue, two muls per head-pair tile
    with base-64 outputs (walrus constrains only INPUT partition bases, and
    only when both inputs are SBUF).
  - emission interleaves each pair's attention with the NEXT pair's
    qk/v/rope chunks: the PE is in-order, so attention-phase bubbles can only
    be filled by placing independent matmuls between them in the stream.
  - per-pair x loads go on the GPSIMD (SWDGE) queue so the latency-critical
    reciprocal-broadcast DMAs never queue behind them on the sync queue.
  - q-bias folded into the psum->sbuf eviction on ACT (Identity + bias AP,
    same act table set as Exp); proj bias via DVE tensor_scalar_add.
  - output projection emitted transposed (free dim = tokens); host
    re-transposes y.
"""
import numpy as np
import ml_dtypes

B, N, C, H, R = 64, 197, 768, 12, 24
D = C // H               # 64
NCORES = 8
BPC = B // NCORES        # images per core
F2 = 2 * N               # 394
F4 = 4 * N               # 788
N0, N1 = 128, N - 128    # token chunks: 128 + 69
VW = 65                  # per-head v block: 64 channels + ones column
VC = 12 * VW             # 780

_cache = {}

SHUF_MASK = list(range(16, 32)) + list(range(0, 16))


def _perm64():
    p = []
    for blk in range(2):
        base = blk * 32
        p += [base + 2 * t for t in range(16)]
        p += [base + 2 * t + 1 for t in range(16)]
    return np.array(p)


def build_program(n_pairs=BPC // 2, repeat=1):
    import concourse.bass as bass
    import concourse.tile as tile
    from concourse import bacc, mybir

    f32, bf16 = mybir.dt.float32, mybir.dt.bfloat16
    AF = mybir.ActivationFunctionType

    nc = bacc.Bacc("TRN2", target_bir_lowering=False, debug=False)

    xt_d = nc.dram_tensor("xt", [n_pairs, C, F2], bf16, kind="ExternalInput")
    wt_d = nc.dram_tensor("wt", [C, 3 * C], bf16, kind="ExternalInput")
    eye_d = nc.dram_tensor("eye", [128, 128], bf16, kind="ExternalInput")
    bq_d = nc.dram_tensor("bq", [128, 6], f32, kind="ExternalInput")
    cs_d = nc.dram_tensor("cs", [2, 128, F4], bf16, kind="ExternalInput")
    erpb_d = nc.dram_tensor("erpb", [12, 128, F2], bf16, kind="ExternalInput")
    projt_d = nc.dram_tensor("projt", [C, C], bf16, kind="ExternalInput")
    projb_d = nc.dram_tensor("projb", [128, 6], f32, kind="ExternalInput")
    y_d = nc.dram_tensor("y", [n_pairs, C, F2], bf16, kind="ExternalOutput")

    from contextlib import ExitStack
    with tile.TileContext(nc) as tc:
        with ExitStack() as stk:
            pool = lambda name, bufs, **kw: stk.enter_context(
                tc.tile_pool(name=name, bufs=bufs, **kw))
            # NOTE: bufs is per-tag. PSUM budget: qkps 2 + vps 1 + psA 1 +
            # psB 1 + aops 2 + yps 1 = 8 banks exactly.
            constp = pool("const", 1)
            xtp = pool("xt", 2)
            qkps = pool("qkps", 2, space="PSUM")
            vps = pool("vps", 1, space="PSUM")
            qkbfp = pool("qkbf", 2)
            ropet = pool("ropet", 2)
            vsbp = pool("vsb", 8)
            scps = pool("scps", 1, space="PSUM")
            yps = pool("yps", 1, space="PSUM")
            probsp = pool("probs", 6)
            aops = pool("aops", 2, space="PSUM")
            rsbp = pool("rsb", 2)
            aotp = pool("aot", 2)
            rbc = pool("rbc", 8)
            ysbp = pool("ysb", 2)
            otp = pool("otp", 6)

            # ---- constants ----
            bq_sb = constp.tile([128, 6], f32, tag="bq")
            nc.sync.dma_start(bq_sb[:], bq_d[:])
            cos_sb = constp.tile([128, F4], bf16, tag="cos")
            nc.sync.dma_start(cos_sb[:], cs_d[0])
            spm_sb = constp.tile([128, F4], bf16, tag="spm")
            nc.sync.dma_start(spm_sb[:], cs_d[1])
            # first pair's inputs, then weights streamed q-cols first so the
            # first q psum groups can start early
            xt_first = []
            for cc in range(6):
                t = xtp.tile([128, F2], bf16, tag=f"xt{cc}", name=f"xtF{cc}")
                nc.sync.dma_start(t[:], xt_d[0, cc * 128:(cc + 1) * 128, :])
                xt_first.append(t)
            wt_all = constp.tile([128, 6 * 3 * C], bf16, tag="wtall")
            for c0, c1 in ((0, C), (C, 2 * C), (2 * C, 3 * C)):
                for cc in range(6):
                    nc.sync.dma_start(
                        wt_all[:, cc * 3 * C + c0:cc * 3 * C + c1],
                        wt_d[cc * 128:(cc + 1) * 128, c0:c1])
            wt_sb = [wt_all[:, cc * 3 * C:(cc + 1) * 3 * C] for cc in range(6)]
            eye_sb = constp.tile([128, 128], bf16, tag="eye")
            nc.sync.dma_start(eye_sb[:], eye_d[:])
            pt_all = constp.tile([128, 6 * C], bf16, tag="ptall")
            erpb_all = constp.tile([128, 12 * F2], bf16, tag="erpball")
            # erpb_sb[hp][jchunk] -> [128, F2] (cols: ph*N + i)
            erpb_sb = [(erpb_all[:, (hp * 2) * F2:(hp * 2 + 1) * F2],
                        erpb_all[:, (hp * 2 + 1) * F2:(hp * 2 + 2) * F2])
                       for hp in range(6)]
            projb_sb = constp.tile([128, 6], f32, tag="pb")

            def load_late_consts():
                nc.sync.dma_start(
                    pt_all[:].rearrange("p (cc j) -> p cc j", cc=6),
                    projt_d.rearrange("(cc p) j -> cc p j", cc=6)
                    .transpose((1, 0, 2)))
                nc.sync.dma_start(
                    erpb_all[:].rearrange("p (g j) -> p g j", g=12),
                    erpb_d.transpose((1, 0, 2)))
                nc.sync.dma_start(projb_sb[:], projb_d[:])

            zr_sb = constp.tile([1, 4], bf16, tag="zr")
            nc.vector.memset(zr_sb[:], 0.0)

            pair_state = {}

            def front_gen(pi):
                """qk/v/rope for pair pi, yielding between chunks so the
                driver can interleave with the previous pair's attention."""
                p = pi % n_pairs
                if pi == 0:
                    xt_sb = xt_first
                    load_late_consts()
                else:
                    xt_sb = []
                    for cc in range(6):
                        t = xtp.tile([128, F2], bf16, tag=f"xt{cc}",
                                     name=f"xt{pi}{cc}")
                        nc.gpsimd.dma_start(
                            t[:], xt_d[p, cc * 128:(cc + 1) * 128, :])
                        xt_sb.append(t)
                yield
                ropes = {}
                for m in range(12):
                    qkt = qkbfp.tile([128, F2], bf16, tag=f"qk{m}",
                                     name=f"qk{pi}{m}")
                    ps = qkps.tile([128, F2], f32, tag="qkps",
                                   padded_shape=[128, 512], name=f"qkp{pi}{m}")
                    for cc in range(6):
                        nc.tensor.matmul(
                            ps[:],
                            lhsT=wt_sb[cc][:, m * 128:(m + 1) * 128],
                            rhs=xt_sb[cc][:],
                            start=(cc == 0), stop=(cc == 5))
                    if m < 6:
                        nc.scalar.activation(qkt[:], ps[:], AF.Identity,
                                             bias=bq_sb[:, m:m + 1])
                    else:
                        nc.scalar.activation(qkt[:], ps[:], AF.Copy)
                    # rope this tile right away
                    qs = ropet.tile([128, F2], bf16, tag="qs", name=f"qs{pi}{m}")
                    nc.vector.stream_shuffle(
                        qs[:].bitcast(f32), qkt[:].bitcast(f32), SHUF_MASK)
                    u = ropet.tile([128, F2], bf16, tag="u", name=f"u{pi}{m}")
                    v = ropet.tile([128, F2], bf16, tag="v", name=f"v{pi}{m}")
                    nc.vector.tensor_mul(u[:], qkt[:], cos_sb[:, 0:F2])
                    nc.vector.tensor_mul(v[:], qs[:], spm_sb[:, 0:F2])
                    nc.vector.tensor_add(qkt[:], u[:], v[:])
                    ropes[m] = qkt
                    yield
                v_sb = []
                for ic in range(2):
                    vts = [vsbp.tile([128, VC], bf16, tag="vsb",
                                     name=f"vsb{pi}{ic}{i}") for i in range(2)]
                    for i in range(2):
                        nc.gpsimd.memset(
                            vts[i][:].rearrange("p (h c) -> p h c", c=VW)
                            [:, :, 64:65], 1.0)
                    for nck, (n_off, n_sz) in enumerate(((0, N0), (N0, N1))):
                        for ch in range(2):
                            ps = vps.tile([128, 384], f32, tag="vps",
                                          padded_shape=[128, 512],
                                          name=f"vps{pi}{ic}{nck}{ch}")
                            for cc in range(6):
                                nc.tensor.matmul(
                                    ps[0:n_sz, :],
                                    lhsT=xt_sb[cc][:, ic * N + n_off:ic * N + n_off + n_sz],
                                    rhs=wt_sb[cc][:, 2 * C + ch * 384:2 * C + (ch + 1) * 384],
                                    start=(cc == 0), stop=(cc == 5))
                            nc.scalar.activation(
                                vts[nck][0:n_sz, ch * 6 * VW:(ch + 1) * 6 * VW]
                                .rearrange("p (h c) -> p h c", c=VW)[:, :, 0:64],
                                ps[0:n_sz, :].rearrange("p (h c) -> p h c", c=64),
                                AF.Copy)
                            yield
                    v_sb.append(vts)
                pair_state[p] = (ropes, v_sb)

            def attn_gen(pi):
                """Scores/attn/normalize/proj for image pair p."""
                p = pi % n_pairs
                ropes, v_sb = pair_state.pop(p)
                aot = aotp.tile([65, 12 * F2], bf16, tag="aot", name=f"aot{p}")
                ot_sb = []
                for hp in range(6):
                    qro = ropes[hp]
                    kro = ropes[hp + 6]
                    ao_t = [None, None]
                    for ic in range(2):
                        qoff = ic * N
                        psA = scps.tile([128, F2], f32, tag="psA",
                                        padded_shape=[128, 512],
                                        name=f"psA{p}{hp}{ic}")
                        psB = scps.tile([128, F2], f32, tag="psB",
                                        padded_shape=[128, 512],
                                        name=f"psB{p}{hp}{ic}")
                        # seed psum with the rel-pos bias via identity
                        # matmuls (also marks the bytes matmul-written)
                        nc.tensor.matmul(
                            psA[:], lhsT=eye_sb[:, 0:128], rhs=erpb_sb[hp][0],
                            start=True, stop=False, skip_group_check=True)
                        nc.tensor.matmul(
                            psB[0:N1, :], lhsT=eye_sb[0:N1, 0:N1],
                            rhs=erpb_sb[hp][1][0:N1, :],
                            start=True, stop=False, skip_group_check=True)
                        for ph in range(2):
                            cr = ph * N
                            qv = qro[ph * 64:(ph + 1) * 64, qoff:qoff + N]
                            if ph == 1:
                                # K=1 spacer (see module docstring)
                                nc.tensor.matmul(
                                    psA[0:1, 0:1], lhsT=zr_sb[0:1, 0:1],
                                    rhs=zr_sb[0:1, 1:2], start=False,
                                    stop=False, skip_group_check=True)
                            nc.tensor.matmul(
                                psA[:, cr:cr + N],
                                lhsT=kro[ph * 64:(ph + 1) * 64, qoff:qoff + 128],
                                rhs=qv, start=False, stop=(ph == 1),
                                skip_group_check=True)
                            if ph == 1:
                                nc.tensor.matmul(
                                    psB[0:1, 0:1], lhsT=zr_sb[0:1, 0:1],
                                    rhs=zr_sb[0:1, 1:2], start=False,
                                    stop=False, skip_group_check=True)
                            nc.tensor.matmul(
                                psB[0:N1, cr:cr + N],
                                lhsT=kro[ph * 64:(ph + 1) * 64, qoff + 128:qoff + N],
                                rhs=qv, start=False, stop=(ph == 1),
                                skip_group_check=True)
                        prA = probsp.tile([128, F2], bf16, tag="prA",
                                          name=f"prA{p}{hp}{ic}")
                        prB = probsp.tile([128, F2], bf16, tag="prB",
                                          name=f"prB{p}{hp}{ic}")
                        nc.scalar.activation(prA[:], psA[:], AF.Exp)
                        nc.scalar.activation(prB[0:N1, :], psB[0:N1, :], AF.Exp)
                        for ph in range(2):
                            h = 2 * hp + ph
                            cr = ph * N
                            if ic == 0:
                                ao_t[ph] = aops.tile(
                                    [65, F2], f32, tag="aops",
                                    padded_shape=[65, 512], name=f"ao{p}{h}")
                            # ones column at v block col 64 accumulates the
                            # softmax denominator into psum row 64 for free
                            nc.tensor.matmul(
                                ao_t[ph][:, ic * N:(ic + 1) * N],
                                lhsT=v_sb[ic][0][:, h * VW:(h + 1) * VW],
                                rhs=prA[:, cr:cr + N],
                                start=(ic == 0), stop=False,
                                skip_group_check=True)
                            nc.tensor.matmul(
                                ao_t[ph][:, ic * N:(ic + 1) * N],
                                lhsT=v_sb[ic][1][0:N1, h * VW:(h + 1) * VW],
                                rhs=prB[0:N1, cr:cr + N],
                                start=False, stop=(ic == 1),
                                skip_group_check=True)
                        yield
                    rbs = []
                    for ph in range(2):
                        h = 2 * hp + ph
                        nc.scalar.activation(
                            aot[:, h * F2:(h + 1) * F2], ao_t[ph][:], AF.Copy)
                        rs = rsbp.tile([1, F2], bf16, tag="rs",
                                       name=f"rs{p}{h}")
                        with nc.allow_low_precision(
                                reason="1/sums in bf16, matches v2 rsmb"):
                            nc.vector.reciprocal(rs[:], ao_t[ph][64:65, :])
                        rb = rbc.tile([64, F2], bf16, tag="rbc",
                                      name=f"rb{p}{h}")
                        nc.sync.dma_start(
                            rb[:],
                            rs[0:1, :].unsqueeze(1)
                            .broadcast_to((1, 64, F2)))
                        rbs.append(rb)
                    t = hp
                    ot = otp.tile([128, F2], bf16, tag="ot", name=f"ot{p}{t}")
                    nc.vector.tensor_mul(
                        ot[0:64, :],
                        aot[0:64, (2 * t) * F2:(2 * t + 1) * F2], rbs[0][:])
                    nc.vector.tensor_mul(
                        ot[64:128, :],
                        aot[0:64, (2 * t + 1) * F2:(2 * t + 2) * F2], rbs[1][:])
                    ot_sb.append(ot)
                    yield

                # ---- output projection, transposed (free dim = tokens) ----
                for oc in range(6):
                    ps = yps.tile([128, F2], f32, tag="yps",
                                  padded_shape=[128, 512], name=f"yps{p}{oc}")
                    for cc in range(6):
                        nc.tensor.matmul(
                            ps[:],
                            lhsT=pt_all[:, cc * C + oc * 128:cc * C + (oc + 1) * 128],
                            rhs=ot_sb[cc][:],
                            start=(cc == 0), stop=(cc == 5))
                    yt = ysbp.tile([128, F2], bf16, tag="ysb",
                                   name=f"yt{p}{oc}")
                    nc.vector.tensor_scalar_add(yt[:], ps[:],
                                                projb_sb[:, oc:oc + 1])
                    nc.sync.dma_start(
                        y_d[p % n_pairs, oc * 128:(oc + 1) * 128, :], yt[:])
                    yield

            total_pairs = repeat * n_pairs
            SENT = object()
            pending = None
            for pi in range(total_pairs):
                fg = front_gen(pi)
                if pending is None:
                    for _ in fg:
                        pass
                else:
                    done_f = done_a = False
                    while not (done_f and done_a):
                        if not done_a:
                            done_a = next(pending, SENT) is SENT
                        if not done_f:
                            done_f = next(fg, SENT) is SENT
                    for _ in fg:
                        pass
                pending = attn_gen(pi)
            for _ in pending:
                pass
    nc.compile()
    return nc


def host_prepare(inputs):
    x = np.asarray(inputs["x"], np.float32)
    qkv_w = np.asarray(inputs["qkv_w"], np.float32)
    scale = D ** -0.5
    Wq = qkv_w[:C] + np.asarray(inputs["lora_q_b"]) @ np.asarray(inputs["lora_q_a"])
    Wk = qkv_w[C:2 * C] + np.asarray(inputs["lora_k_b"]) @ np.asarray(inputs["lora_k_a"])
    Wv = qkv_w[2 * C:] + np.asarray(inputs["lora_v_b"]) @ np.asarray(inputs["lora_v_a"])
    p64 = _perm64()
    perm = (np.arange(H)[:, None] * D + p64[None, :]).ravel()
    Wq_de = (Wq * scale)[perm]
    bq_de = (np.asarray(inputs["q_bias"], np.float32) * scale)[perm]
    Wk_de = Wk[perm]
    wt = np.ascontiguousarray(
        np.concatenate([Wq_de, Wk_de, Wv], 0).T).astype(ml_dtypes.bfloat16)

    eye = np.eye(128, dtype=np.float32).astype(ml_dtypes.bfloat16)
    bq = np.ascontiguousarray(bq_de.reshape(6, 128).T)

    cos_f = np.ones((N, D), np.float32)
    cos_f[1:] = np.asarray(inputs["rope_cos"], np.float32)
    sin_f = np.zeros((N, D), np.float32)
    sin_f[1:] = np.asarray(inputs["rope_sin"], np.float32)
    cos_de = np.ascontiguousarray(cos_f[:, p64].T)
    spm = np.ascontiguousarray(sin_f[:, p64].T)
    for blk in range(2):
        spm[blk * 32:blk * 32 + 16] *= -1.0
    cs = np.stack([
        np.tile(np.vstack([cos_de, cos_de]), (1, 4)),
        np.tile(np.vstack([spm, spm]), (1, 4)),
    ]).astype(ml_dtypes.bfloat16)

    rel_table = np.asarray(inputs["rel_table"], np.float32)
    rel_index = np.asarray(inputs["rel_index"])
    rpb = rel_table[rel_index.reshape(-1)].reshape(N, N, H)
    rpbT = rpb.transpose(2, 1, 0)  # [h, j, i] (raw; added to scores in psum)
    # erpb[(hp, jchunk), j_local, ph*N + i]
    erpb = np.zeros((12, 128, F2), np.float32)
    for hp in range(6):
        for jc in range(2):
            jn = 128 if jc == 0 else N1
            for ph in range(2):
                erpb[hp * 2 + jc, 0:jn, ph * N:(ph + 1) * N] = \
                    rpbT[2 * hp + ph, jc * 128:jc * 128 + jn, :]
    erpb = erpb.astype(ml_dtypes.bfloat16)

    proj_w = np.asarray(inputs["proj_w"], np.float32)
    projt = np.ascontiguousarray(proj_w.T).astype(ml_dtypes.bfloat16)
    projb_full = (np.asarray(inputs["proj_b"], np.float32)
                  + proj_w @ np.asarray(inputs["v_bias"], np.float32))
    projb = np.ascontiguousarray(projb_full.reshape(6, 128).T)

    xt = x.transpose(0, 2, 1)  # [B, C, N]
    xt_pairs = np.ascontiguousarray(
        xt.reshape(B // 2, 2, C, N).transpose(0, 2, 1, 3)
        .reshape(B // 2, C, 2 * N)).astype(ml_dtypes.bfloat16)

    shared = dict(wt=wt, bq=bq, cs=cs, erpb=erpb,
                  projt=projt, projb=projb, eye=eye)
    per_core = []
    ppc = BPC // 2
    for c in range(NCORES):
        m = dict(shared)
        m["xt"] = np.ascontiguousarray(xt_pairs[c * ppc:(c + 1) * ppc])
        per_core.append(m)
    return per_core


def postprocess(y_all):
    """y_all: [..., C, F2] pair-major; returns [n_images, N, C]."""
    y = np.asarray(y_all, np.float32).reshape(-1, C, 2, N)
    return np.ascontiguousarray(
        y.transpose(0, 2, 3, 1).reshape(-1, N, C))


def kernel(**inputs):
    from concourse.bass_utils import run_bass_kernel_spmd
    in_maps = host_prepare(inputs)
    if "nc" not in _cache:
        _cache["nc"] = build_program()
    nc = _cache["nc"]
    res = run_bass_kernel_spmd(nc, in_maps, list(range(NCORES))).results
    y = np.concatenate([res[c]["y"] for c in range(NCORES)], 0)
    return postprocess(y)
